# revision 1
# baseline (speedup 1.0000x reference)
"""nn_AdapFilter3d Trainium2 kernel — 8-core SPMD (data-parallel over (B,C)).

out[b,c,z,y,x] = sum_{i,j,k} pad(input)[b,c,z+i-1,y+j-1,x+k-1] * F[b,c,z,y,x,i,j,k]

Strategy (per NeuronCore: 4 of the 32 independent (b,c) slices = 2 slice-pairs;
partitions p = 64*s + y; free dims carry (z, x) densely):

  - y-shift via the accumulating matmul, not via extra input copies: with
    G_t = x * F_t^(pre-shifted by -dy on host), the PSUM accumulation
    S_dy.T @ G_t equals shift_dy(x) * F_t. The three stationaries S_{-1}, I,
    S_{+1} (boundary rows zeroed, block-diagonal per slice) replace the
    baseline's three y-shifted HBM copies of the input — input is read once.
  - x/z shifts stay free-dim offsets into one padded dense (z,x) row; edge-tap
    F values are zeroed host-side (exact: those reference contributions are
    zero through the zero padding).
  - F is laid out j-major tap-contiguous per partition ([chunk][j,i,k][z][x]),
    so each (j, chunk) needs ONE fused VectorE multiply over a (i:3,k:3,e:512)
    overlapping access pattern of the input window — 3 DVE instructions per
    chunk instead of 27, all operands innermost-contiguous bf16 (2x mode).
  - Per chunk: 27 accumulating TensorE matmuls (9 per stationary) into one
    PSUM bank; ScalarE evicts to bf16; output DMA rides the gpsimd queue so
    it never head-of-line-blocks F loads on the sync/scalar rings.
  - F streams as 6 sub-DMAs per chunk (per j-group, split across the two
    HWDGE rings by partition half) so the first multiply starts after ~1MB.

Measured on 8xTRN2 (neuron-profile, SPMD all cores): ~109-111us HW exec
(best 105.6us; baseline 113.8us), L2 rel err ~3.1e-3 (fp32 reference; bf16
input rounding dominates). The kernel sits at the activity-throttled DMA
roofline: ~29.5MB HBM read/core at the ~270-400GB/s duty-limited rate; fill
is ~15us (was 37us) and drain ~12us. 64-partition DMA splits measured ~30%
slower per byte than whole-128-partition transfers and are avoided.

Self-contained: hardcodes shapes from the problem spec; needs only the
concourse/axon environment on sys.path.
"""

import time

import numpy as np

import bass_rust
import concourse.bacc as bacc
import concourse.tile as tile
from concourse import mybir
from concourse.bass_utils import run_bass_kernel_spmd

B, C, D, H, W = 2, 16, 32, 64, 64
BC = B * C
TAPS = 27
N_CORES = 8
S = BC // N_CORES  # 4 slices per core
PAIRS = S // 2  # 2
ZC = 8  # z planes per chunk
NCHUNK = D // ZC  # 4
FD = ZC * W  # 512
JFD = 9 * FD  # 4608 (one j-group)
CFD = TAPS * FD  # 13824 (one chunk of F)
DW = D * W  # 2048 dense (z,x) elements per (slice, y)
FRONT = 65  # zero pad around the dense (z,x) block (>= W+1)
XPLEN = FRONT + DW + FRONT

F32 = mybir.dt.float32
IO_DT = mybir.dt.bfloat16


def _overlap_ap(tile_ap, start, dims):
    """AP on tile_ap's tensor at element offset `start` with custom free dims
    [[stride, num], ...] (keeps the tile's partition dim)."""
    return bass_rust.AP(tile_ap.tensor, start, [list(tile_ap.ap[0])] + dims)


def _build():
    nc = bacc.Bacc()
    x_ext = nc.declare_dram_parameter("input", [PAIRS, 128, XPLEN], IO_DT, isOutput=False)
    f_ext = nc.declare_dram_parameter("F", [PAIRS, 128, NCHUNK * CFD], IO_DT, isOutput=False)
    s_ext = nc.declare_dram_parameter("stat", [128, 3 * 128], IO_DT, isOutput=False)
    out_ext = nc.declare_dram_parameter("out", [PAIRS, 128, NCHUNK * FD], IO_DT, isOutput=True)

    with tile.TileContext(nc) as tc:
        with (
            tc.tile_pool(name="const", bufs=1) as cpool,
            tc.tile_pool(name="xp", bufs=2) as xpool,
            tc.tile_pool(name="fp", bufs=3) as fpool,
            tc.tile_pool(name="prod", bufs=6) as ppool,
            tc.tile_pool(name="osb", bufs=2) as opool,
            tc.tile_pool(name="ps", bufs=6, space="PSUM") as pspool,
        ):
            st = cpool.tile([128, 3 * 128], IO_DT)

            # x/st ride the scalar queue up front (it otherwise carries only
            # evict+out, so nothing compute-blocked sits ahead of them) and
            # leave the sync/gpsimd rings purely to the F stream
            xps = []
            for pair in range(PAIRS):
                xp = xpool.tile([128, XPLEN], IO_DT, tag="xp")
                nc.scalar.dma_start(xp[:, :], x_ext[pair, :, :])
                xps.append(xp)
            nc.scalar.dma_start(st[:], s_ext[:])

            for pair in range(PAIRS):
                xp = xps[pair]
                for ch in range(NCHUNK):
                    ft = fpool.tile([128, CFD], IO_DT, tag="ft")
                    # Whole-128-partition single-queue DMAs: measured ~30%
                    # faster per byte than any 64-partition split (DRAM-side
                    # streaming locality). Rings alternate by chunk parity
                    # (chunk 0 on gpsimd, parallel to x/st on sync). The first
                    # two and last chunks are j-split (still 128-partition) so
                    # the pipeline fills and drains at j-group granularity.
                    eng = nc.gpsimd if ch % 2 == 0 else nc.sync
                    first = pair == 0 and ch == 0
                    lastc = pair == PAIRS - 1 and ch == NCHUNK - 1
                    if first:
                        # very first j-group streams in 3-tap slices so the
                        # first multiply starts after ~0.4MB instead of 1.2MB
                        for lo in range(0, JFD, 3 * FD):
                            eng.dma_start(
                                ft[:, lo : lo + 3 * FD],
                                f_ext[pair, :, ch * CFD + lo : ch * CFD + lo + 3 * FD],
                            )
                        for j in range(1, 3):
                            lo, hi = j * JFD, (j + 1) * JFD
                            eng.dma_start(
                                ft[:, lo:hi], f_ext[pair, :, ch * CFD + lo : ch * CFD + hi]
                            )
                    else:
                        splits = 3 if lastc else 1
                        step = CFD // splits
                        for j in range(splits):
                            lo, hi = j * step, (j + 1) * step
                            eng.dma_start(
                                ft[:, lo:hi], f_ext[pair, :, ch * CFD + lo : ch * CFD + hi]
                            )
                    psum = pspool.tile([128, FD], F32, tag="ps")
                    for j in range(3):
                        # the first chunk's j=0 multiply is split into 3-tap
                        # slices matching its DMA granularity (fill only)
                        nsub = 3 if (first and j == 0) else 1
                        prod = ppool.tile([128, JFD], IO_DT, tag="prod")
                        for sub in range(nsub):
                            ilo = sub * (3 // nsub) if nsub == 3 else 0
                            isz = 1 if nsub == 3 else 3
                            # x window AP overlaps itself (stride W on i, 1 on k)
                            xap = _overlap_ap(
                                xp[:],
                                FRONT + (ch * ZC + ilo - 1) * W - 1,
                                [[W, isz], [1, 3], [1, FD]],
                            )
                            off = j * JFD + ilo * 3 * FD
                            fap = ft[:, off : off + isz * 3 * FD].rearrange(
                                "p (i k e) -> p i k e", i=isz, k=3
                            )
                            pap = prod[:, ilo * 3 * FD : (ilo + isz) * 3 * FD].rearrange(
                                "p (i k e) -> p i k e", i=isz, k=3
                            )
                            nc.vector.tensor_mul(pap, xap, fap)
                        for t9 in range(9):
                            T = j * 9 + t9
                            nc.tensor.matmul(
                                psum[:],
                                st[:, j * 128 : (j + 1) * 128],
                                prod[:, t9 * FD : (t9 + 1) * FD],
                                start=(T == 0),
                                stop=(T == TAPS - 1),
                            )
                    # eviction + output writes live on the scalar queue ONLY:
                    # their waits on the matmul chain cannot block F loads
                    # (sync/gpsimd queues) or the next chunk's multiplies
                    osb = opool.tile([128, FD], IO_DT, tag="osb")
                    nc.scalar.copy(osb[:], psum[:])
                    nc.scalar.dma_start(out_ext[pair, :, ch * FD : (ch + 1) * FD], osb[:])
    nc.compile()
    return nc


_NC_CACHE = {}


def _host_inputs(input, F):
    """FULL inputs -> per-core in_maps with the kernel's layouts."""
    io_np = mybir.dt.np(IO_DT)
    # x dense rows: xs[bc, y, FRONT + z*W + x]
    xs = np.zeros((BC, H, XPLEN), dtype=io_np)
    xs[:, :, FRONT : FRONT + DW] = (
        input.reshape(BC, D, H, W).transpose(0, 2, 1, 3).reshape(BC, H, DW).astype(io_np)
    )
    xs = xs.reshape(BC // 2, 128, XPLEN)

    # F pre-shifted along y by -dy per j, j-major tap order, edge taps zeroed
    base = np.ascontiguousarray(
        F.reshape(BC, D, H, W, 3, 3, 3).transpose(0, 2, 5, 4, 6, 1, 3)
    )  # [bc, y, j, i, k, z, x]
    Hs = np.zeros_like(base)
    Hs[:, : H - 1, 0] = base[:, 1:, 0]
    Hs[:, :, 1] = base[:, :, 1]
    Hs[:, 1:, 2] = base[:, : H - 1, 2]
    Hs[:, :, :, :, 0, :, 0] = 0
    Hs[:, :, :, :, 2, :, W - 1] = 0
    Hs[:, :, :, 0, :, 0, :] = 0
    Hs[:, :, :, 2, :, D - 1, :] = 0
    fs = (
        Hs.reshape(BC, H, TAPS, NCHUNK, ZC, W)
        .transpose(0, 1, 3, 2, 4, 5)  # [bc, y, ch, T, zc, x]
        .reshape(BC // 2, 128, NCHUNK * CFD)
        .astype(io_np)
    )

    # stationaries: st[kk, j*128+m] = 1 iff kk == m + (j-1), same 64-block
    st = np.zeros((128, 3, 128), dtype=np.float32)
    for j in range(3):
        Sj = np.eye(128, k=-(j - 1), dtype=np.float32)
        Sj[0:64, 64:128] = 0
        Sj[64:128, 0:64] = 0
        st[:, j, :] = Sj
    st = st.reshape(128, 3 * 128).astype(io_np)

    return [
        {
            "input": xs[c * PAIRS : (c + 1) * PAIRS],
            "F": fs[c * PAIRS : (c + 1) * PAIRS],
            "stat": st,
        }
        for c in range(N_CORES)
    ]


def kernel(input: np.ndarray, F: np.ndarray) -> np.ndarray:
    input = np.asarray(input)
    F = np.asarray(F)
    assert input.shape == (B, C, D, H, W), input.shape
    assert F.shape == (B, C, D, H, W, 3, 3, 3), F.shape

    if "nc" not in _NC_CACHE:
        _NC_CACHE["nc"] = _build()
    nc = _NC_CACHE["nc"]

    in_maps = _host_inputs(input, F)
    # the fleet occasionally throws transient NRT_EXEC_UNIT_UNRECOVERABLE
    # device errors (observed in dev, cleared on retry)
    last_err = None
    for _attempt in range(3):
        try:
            res = run_bass_kernel_spmd(nc, in_maps, core_ids=list(range(N_CORES)))
            break
        except Exception as e:  # noqa: BLE001
            last_err = e
            time.sleep(2.0)
    else:
        raise last_err
    out = np.concatenate(
        [np.asarray(res.results[c]["out"], dtype=np.float32) for c in range(N_CORES)],
        axis=0,
    )  # [BC/2, 128, NCHUNK*FD]
    out = (
        out.reshape(BC // 2, 2, H, NCHUNK, ZC, W)
        .transpose(0, 1, 3, 4, 2, 5)  # [pair, s, ch, zc, y, x]
        .reshape(B, C, D, H, W)
        .astype(np.float32)
    )
    return np.ascontiguousarray(out)

